# revision 14
# baseline (speedup 1.0000x reference)
"""Trainium2 Bass kernel for nn_Attention_81750407512209.

Full attention: out = softmax((x Wq)(x Wk)^T / sqrt(128)) @ (x Wv)
B=8 batches sharded 1:1 onto 8 NeuronCores (data parallel, weights replicated).

Per-core design (N=4096 ctx, D=128):
  - x^T via PE transpose; Q^T/K^T projections computed in float32r
    (~1.5e-4 matmul rel err measured on silicon) then stored bf16;
    1/sqrt(128) folded into Wq.  Scores matmul runs bf16 (2-byte moving
    operand streams at 1 cyc/row vs ~2.4 for 4-byte) - measured end-to-end
    rel err ~2e-3 vs the f32 reference.
  - Scores per 128-row q-tile in PSUM chunks (1536,1536,1024) - pool of
    two 3-bank slots + the 1024 chunk reuses a freed slot.
  - Row max via DVE reduce_max(negate=True) per chunk.
  - "Flash-lite" softmax: chunks 0,1 exponentiated with bias -max(c0,c1),
    chunk 2 with the full row -max; single PSUM rescale of the AV
    accumulator by gamma = exp(max01 - max) between AV kv-halves.
  - P = exp(S + bias) on ScalarE, PSUM -> SBUF bf16.
  - P^T via wide xbar DMA transposes ([128,2048] -> [128,16,128] batched
    block transpose) on the sync HWDGE engine only (xbar is a serialized
    resource; dual-engine issue corrupts data - measured).
  - AV: 32 bf16 matmuls lhsT=P^T tile [kv,q], rhs=V tile augmented with a
    ones column -> row sums accumulate in PSUM col 128.  Normalize with
    DVE reciprocal + ScalarE copy*scale.
  - Software pipelined: q-tile i-1's AV/normalize emitted interleaved with
    q-tile i's score work so PE is never blocked on the softmax chain.
"""

import numpy as np
from contextlib import ExitStack

import concourse.bass as bass
import concourse.tile as tile
from concourse import bacc, mybir
from concourse.bass_utils import run_bass_kernel_spmd
from concourse.masks import make_identity

F32 = mybir.dt.float32
F32R = mybir.dt.float32r
BF16 = mybir.dt.bfloat16
AX = mybir.AxisListType.X
OP = mybir.AluOpType
AF = mybir.ActivationFunctionType

B, N, D = 8, 4096, 128
NT = N // 128                    # 32 kv/q tiles
CHUNKS = (1536, 1536, 1024)      # score chunks; c0,c1 share bias m01
SCALE = 1.0 / np.sqrt(np.float32(D))
RESCALE_T = (CHUNKS[0] + CHUNKS[1]) // 128   # kv-tile where gamma applies (16)


def build_attention(nc: bacc.Bacc):
    x = nc.dram_tensor("x", [N, D], F32, kind="ExternalInput").ap()
    wq = nc.dram_tensor("w_query", [D, D], F32, kind="ExternalInput").ap()
    wk = nc.dram_tensor("w_key", [D, D], F32, kind="ExternalInput").ap()
    wv = nc.dram_tensor("w_value", [D, D], F32, kind="ExternalInput").ap()
    out = nc.dram_tensor("out", [N, D], F32, kind="ExternalOutput").ap()

    with tile.TileContext(nc) as tc, ExitStack() as ctx:
        consts = ctx.enter_context(tc.tile_pool(name="consts", bufs=1))
        big = ctx.enter_context(tc.tile_pool(name="big", bufs=1))
        xin = ctx.enter_context(tc.tile_pool(name="xin", bufs=8))
        pbuf = ctx.enter_context(tc.tile_pool(name="pbuf", bufs=3))
        stats = ctx.enter_context(tc.tile_pool(name="stats", bufs=6))
        ostage = ctx.enter_context(tc.tile_pool(name="ostage", bufs=4))

        ident = consts.tile([128, 128], F32, name="ident")
        make_identity(nc, ident[:])

        wq_st = consts.tile([128, 128], F32, name="wq_st")
        wk_st = consts.tile([128, 128], F32, name="wk_st")
        wv_st = consts.tile([128, 128], F32, name="wv_st")
        nc.sync.dma_start(wq_st[:], wq[:])
        nc.sync.dma_start(wk_st[:], wk[:])
        nc.sync.dma_start(wv_st[:], wv[:])
        wq_r = consts.tile([128, 128], F32R, name="wq_r")
        wk_r = consts.tile([128, 128], F32R, name="wk_r")
        wv_r = consts.tile([128, 128], F32R, name="wv_r")
        nc.vector.tensor_scalar_mul(wq_r[:], wq_st[:], float(SCALE))
        nc.vector.tensor_copy(wk_r[:], wk_st[:])
        nc.vector.tensor_copy(wv_r[:], wv_st[:])

        xT = big.tile([128, N], F32R, name="xT")
        kT = big.tile([128, N], BF16, name="kT")
        qT = big.tile([128, N], BF16, name="qT")
        vaug = big.tile([128, NT, 129], BF16, name="vaug")
        nc.gpsimd.memset(vaug[:, :, 128:129], 1.0)

        # ---- prologue: x^T, projections (scoped PSUM pool) ----
        with tc.tile_pool(name="ps_pro", bufs=2, space="PSUM") as ps_pro:
            for c in range(N // 512):
                sl = slice(c * 512, (c + 1) * 512)
                for u in range(4):
                    i = c * 4 + u
                    xt = xin.tile([128, 128], F32, tag="xt", name="xt")
                    nc.gpsimd.dma_start(xt[:], x[i * 128:(i + 1) * 128, :])
                    ps = ps_pro.tile([128, 128], F32, tag="xtp", name="xtp")
                    nc.tensor.transpose(ps[:], xt[:], ident[:])
                    if i % 2 == 0:
                        nc.vector.tensor_copy(xT[:, i * 128:(i + 1) * 128], ps[:])
                    else:
                        nc.scalar.copy(xT[:, i * 128:(i + 1) * 128], ps[:])
                pk = ps_pro.tile([128, 512], F32, tag="proj", name="pk")
                nc.tensor.matmul(pk[:], wk_r[:], xT[:, sl], start=True, stop=True)
                nc.vector.tensor_copy(kT[:, sl], pk[:])
                pq = ps_pro.tile([128, 512], F32, tag="proj", name="pq")
                nc.tensor.matmul(pq[:], wq_r[:], xT[:, sl], start=True, stop=True)
                nc.scalar.copy(qT[:, sl], pq[:])
                for u in range(4):
                    i = c * 4 + u
                    pv = ps_pro.tile([128, 128], F32, tag="vproj", name="pv")
                    nc.tensor.matmul(
                        pv[:], xT[:, i * 128:(i + 1) * 128], wv_r[:],
                        start=True, stop=True,
                    )
                    nc.scalar.copy(vaug[:, i, 0:128], pv[:])

        # ---- main loop pools: 2x3-bank score slots + 2x1-bank AV accum ----
        ps_s = ctx.enter_context(tc.tile_pool(name="ps_s", bufs=2, space="PSUM"))
        ps_av = ctx.enter_context(tc.tile_pool(name="ps_av", bufs=2, space="PSUM"))

        def score_chunk(qsl, off, width):
            s = ps_s.tile([128, CHUNKS[0]], F32, tag="sh", name="sh")
            for k in range(width // 512):
                nc.tensor.matmul(
                    s[:, k * 512:(k + 1) * 512],
                    qsl,
                    kT[:, off + k * 512: off + (k + 1) * 512],
                    start=True,
                    stop=True,
                )
            return s

        def negmax(s, width, tg, pieces=1):
            if pieces == 1:
                nm = stats.tile([128, 1], F32, tag=tg, name="nm")
                nc.vector.reduce_max(nm[:], s[:, 0:width], axis=AX, negate=True)
                return nm
            w = width // pieces
            parts = []
            for p in range(pieces):
                pm = stats.tile([128, 1], F32, tag=f"{tg}p{p}", name="pm")
                nc.vector.reduce_max(
                    pm[:], s[:, p * w:(p + 1) * w], axis=AX, negate=True
                )
                parts.append(pm)
            nm = parts[0]
            for p in range(1, pieces):
                acc = stats.tile([128, 1], F32, tag=f"{tg}a{p}", name="acc")
                nc.vector.tensor_tensor(acc[:], nm[:], parts[p][:], op=OP.min)
                nm = acc
            return nm

        # chunk c0 is exponentiated with its own max (-n0), c1 with -max(c0,c1),
        # c2 with the full row max; AV rescales by gam_a (after kv-tiles 0..11)
        # and gam_b (after kv-tiles 12..23) restore a common exp(-max) scale.
        T_A = CHUNKS[0] // 128   # 12
        T_B = RESCALE_T          # 24
        t1 = None  # (PT, gama, gamb, j): awaiting AVa/AVb
        t2 = None  # (PT, gamb, av, j):   awaiting gb-rescale + tail + norm
        for i in range(NT + 2):
            # A: tile i - first two score chunks and their maxes
            if i < NT:
                qsl = qT[:, i * 128:(i + 1) * 128]
                P = pbuf.tile([128, N], BF16, tag="P", name="P")
                PT = pbuf.tile([128, NT, 128], BF16, tag="PT", name="PT")
                s0 = score_chunk(qsl, 0, CHUNKS[0])
                s1 = score_chunk(qsl, CHUNKS[0], CHUNKS[1])
                n0 = negmax(s0, CHUNKS[0], "n0")
                n1 = negmax(s1, CHUNKS[1], "n1")
                b01 = stats.tile([128, 1], F32, tag="b01", name="b01")
                nc.vector.tensor_tensor(b01[:], n0[:], n1[:], op=OP.min)

            if i < NT:
                for _w in range(2):
                    nc.tensor.ldweights(kT[:, 0:128])

            # B1: tile i-1 - AV over kv-tiles 0..11 (exp(-m0)-scaled)
            if t1 is not None:
                PT1, gama1, gamb1, j1 = t1
                av1 = ps_av.tile([128, 129], F32, tag="av", name="av")
                for t in range(T_A):
                    nc.tensor.matmul(
                        av1[:], PT1[:, t, :], vaug[:, t, :],
                        start=(t == 0), stop=False,
                    )

            # C: tile i-2 - gam_b rescale (ScalarE; inputs one iteration old),
            # AV tail, reciprocal
            if t2 is not None:
                PT2, gamb2, av2, j2 = t2
                nc.scalar.activation(av2[:], av2[:], AF.Copy, bias=0.0, scale=gamb2[:])
                for t in range(T_B, NT):
                    nc.tensor.matmul(
                        av2[:], PT2[:, t, :], vaug[:, t, :],
                        start=False, stop=(t == NT - 1),
                    )
                linv = stats.tile([128, 1], F32, tag="linv", name="linv")
                nc.vector.reciprocal(linv[:], av2[:, 128:129])

            # D1: tile i - exp of c0 (own bias), last chunk + stats + gammas
            if i < NT:
                nc.scalar.activation(P[:, 0:CHUNKS[0]], s0[:], AF.Exp, bias=n0[:])
                off2 = CHUNKS[0] + CHUNKS[1]
                s2 = score_chunk(qsl, off2, CHUNKS[2])
                n2 = negmax(s2, CHUNKS[2], "n2", pieces=2)
                bias = stats.tile([128, 1], F32, tag="bias", name="bias")
                nc.vector.tensor_tensor(bias[:], b01[:], n2[:], op=OP.min)
                gina = stats.tile([128, 1], F32, tag="gina", name="gina")
                nc.vector.tensor_tensor(gina[:], b01[:], n0[:], op=OP.subtract)
                gama = stats.tile([128, 1], F32, tag="gama", name="gama")
                nc.scalar.activation(gama[:], gina[:], AF.Exp)
                ginb = stats.tile([128, 1], F32, tag="ginb", name="ginb")
                nc.vector.tensor_tensor(ginb[:], bias[:], b01[:], op=OP.subtract)
                gamb = stats.tile([128, 1], F32, tag="gamb", name="gamb")
                nc.scalar.activation(gamb[:], ginb[:], AF.Exp)

            # B2: tile i-1 - gam_a rescale then AV over kv-tiles 12..23
            if t1 is not None:
                nc.scalar.activation(av1[:], av1[:], AF.Copy, bias=0.0, scale=gama1[:])
                for t in range(T_A, T_B):
                    nc.tensor.matmul(
                        av1[:], PT1[:, t, :], vaug[:, t, :],
                        start=False, stop=False,
                    )

            if i < NT:
                for _w in range(2):
                    nc.tensor.ldweights(kT[:, 0:128])

            # D2: tile i - exps of c1/c2, xbar transposes
            if i < NT:
                nc.sync.dma_start_transpose(
                    PT[:, 0:T_A, :], P[:, 0:T_A * 128]
                )
                nc.scalar.activation(
                    P[:, CHUNKS[0]:off2], s1[:, 0:CHUNKS[1]], AF.Exp, bias=b01[:]
                )
                nc.sync.dma_start_transpose(
                    PT[:, T_A:T_B, :], P[:, T_A * 128:T_B * 128]
                )
                nc.scalar.activation(
                    P[:, off2:N], s2[:, 0:CHUNKS[2]], AF.Exp, bias=bias[:]
                )
                nc.sync.dma_start_transpose(
                    PT[:, T_B:NT, :], P[:, T_B * 128:N]
                )

            # E: tile i-2 - normalize and store
            if t2 is not None:
                ost = ostage.tile([128, 128], F32, tag="ost", name="ost")
                nc.scalar.activation(
                    ost[:], av2[:, 0:128], AF.Copy, bias=0.0, scale=linv[:]
                )
                nc.gpsimd.dma_start(out[j2 * 128:(j2 + 1) * 128, :], ost[:])

            t2 = (t1[0], t1[2], av1, t1[3]) if t1 is not None else None
            t1 = (PT, gama, gamb, i) if i < NT else None

    nc.compile()
    return nc


_NC_CACHE = {}


def _get_nc():
    if "nc" not in _NC_CACHE:
        nc = bacc.Bacc("TRN2", target_bir_lowering=False, debug=False, num_devices=B)
        _NC_CACHE["nc"] = build_attention(nc)
    return _NC_CACHE["nc"]


def kernel(x, w_query, w_key, w_value, _trace=False):
    x = np.ascontiguousarray(np.asarray(x, dtype=np.float32))
    w_query = np.ascontiguousarray(np.asarray(w_query, dtype=np.float32))
    w_key = np.ascontiguousarray(np.asarray(w_key, dtype=np.float32))
    w_value = np.ascontiguousarray(np.asarray(w_value, dtype=np.float32))
    nc = _get_nc()
    in_maps = [
        {"x": x[b], "w_query": w_query, "w_key": w_key, "w_value": w_value}
        for b in range(B)
    ]
    res = run_bass_kernel_spmd(nc, in_maps, core_ids=list(range(B)), trace=_trace)
    out_full = np.stack([res.results[b]["out"] for b in range(B)])
    if _trace:
        kernel.last_exec_time_ns = res.exec_time_ns
    return out_full


# revision 15
# speedup vs baseline: 1.0143x; 1.0143x over previous
"""Trainium2 Bass kernel for nn_Attention_81750407512209.

Full attention: out = softmax((x Wq)(x Wk)^T / sqrt(128)) @ (x Wv)
B=8 batches sharded 1:1 onto 8 NeuronCores (data parallel, weights replicated).

Per-core design (N=4096 ctx, D=128):
  - x^T via PE transpose; Q^T/K^T projections computed in float32r
    (~1.5e-4 matmul rel err measured on silicon) then stored bf16;
    1/sqrt(128) folded into Wq.  Scores matmul runs bf16 (2-byte moving
    operand streams at 1 cyc/row vs ~2.4 for 4-byte) - measured end-to-end
    rel err ~2e-3 vs the f32 reference.
  - Scores per 128-row q-tile in PSUM chunks (1536,1536,1024) - pool of
    two 3-bank slots + the 1024 chunk reuses a freed slot.
  - Row max via DVE reduce_max(negate=True) per chunk.
  - "Flash-lite" softmax: chunks 0,1 exponentiated with bias -max(c0,c1),
    chunk 2 with the full row -max; single PSUM rescale of the AV
    accumulator by gamma = exp(max01 - max) between AV kv-halves.
  - P = exp(S + bias) on ScalarE, PSUM -> SBUF bf16.
  - P^T via wide xbar DMA transposes ([128,2048] -> [128,16,128] batched
    block transpose) on the sync HWDGE engine only (xbar is a serialized
    resource; dual-engine issue corrupts data - measured).
  - AV: 32 bf16 matmuls lhsT=P^T tile [kv,q], rhs=V tile augmented with a
    ones column -> row sums accumulate in PSUM col 128.  Normalize with
    DVE reciprocal + ScalarE copy*scale.
  - Software pipelined: q-tile i-1's AV/normalize emitted interleaved with
    q-tile i's score work so PE is never blocked on the softmax chain.
"""

import numpy as np
from contextlib import ExitStack

import concourse.bass as bass
import concourse.tile as tile
from concourse import bacc, mybir
from concourse.bass_utils import run_bass_kernel_spmd
from concourse.masks import make_identity

F32 = mybir.dt.float32
F32R = mybir.dt.float32r
BF16 = mybir.dt.bfloat16
AX = mybir.AxisListType.X
OP = mybir.AluOpType
AF = mybir.ActivationFunctionType

B, N, D = 8, 4096, 128
NT = N // 128                    # 32 kv/q tiles
CHUNKS = (1536, 1536, 1024)      # score chunks; c0,c1 share bias m01
SCALE = 1.0 / np.sqrt(np.float32(D))
RESCALE_T = (CHUNKS[0] + CHUNKS[1]) // 128   # kv-tile where gamma applies (16)


def build_attention(nc: bacc.Bacc):
    x = nc.dram_tensor("x", [N, D], F32, kind="ExternalInput").ap()
    wq = nc.dram_tensor("w_query", [D, D], F32, kind="ExternalInput").ap()
    wk = nc.dram_tensor("w_key", [D, D], F32, kind="ExternalInput").ap()
    wv = nc.dram_tensor("w_value", [D, D], F32, kind="ExternalInput").ap()
    out = nc.dram_tensor("out", [N, D], F32, kind="ExternalOutput").ap()

    with tile.TileContext(nc) as tc, ExitStack() as ctx:
        consts = ctx.enter_context(tc.tile_pool(name="consts", bufs=1))
        big = ctx.enter_context(tc.tile_pool(name="big", bufs=1))
        xin = ctx.enter_context(tc.tile_pool(name="xin", bufs=8))
        pbuf = ctx.enter_context(tc.tile_pool(name="pbuf", bufs=3))
        stats = ctx.enter_context(tc.tile_pool(name="stats", bufs=6))
        ostage = ctx.enter_context(tc.tile_pool(name="ostage", bufs=4))

        ident = consts.tile([128, 128], F32, name="ident")
        make_identity(nc, ident[:])

        wq_st = consts.tile([128, 128], F32, name="wq_st")
        wk_st = consts.tile([128, 128], F32, name="wk_st")
        wv_st = consts.tile([128, 128], F32, name="wv_st")
        nc.sync.dma_start(wq_st[:], wq[:])
        nc.sync.dma_start(wk_st[:], wk[:])
        nc.sync.dma_start(wv_st[:], wv[:])
        wq_r = consts.tile([128, 128], F32R, name="wq_r")
        wk_r = consts.tile([128, 128], F32R, name="wk_r")
        wv_r = consts.tile([128, 128], F32R, name="wv_r")
        nc.vector.tensor_scalar_mul(wq_r[:], wq_st[:], float(SCALE))
        nc.vector.tensor_copy(wk_r[:], wk_st[:])
        nc.vector.tensor_copy(wv_r[:], wv_st[:])

        xT = big.tile([128, N], F32R, name="xT")
        kT = big.tile([128, N], BF16, name="kT")
        qT = big.tile([128, N], BF16, name="qT")
        vaug = big.tile([128, NT, 129], BF16, name="vaug")
        nc.gpsimd.memset(vaug[:, :, 128:129], 1.0)

        # ---- prologue: x^T, projections (scoped PSUM pool) ----
        with tc.tile_pool(name="ps_pro", bufs=2, space="PSUM") as ps_pro:
            for c in range(N // 512):
                sl = slice(c * 512, (c + 1) * 512)
                for u in range(4):
                    i = c * 4 + u
                    xt = xin.tile([128, 128], F32, tag="xt", name="xt")
                    nc.gpsimd.dma_start(xt[:], x[i * 128:(i + 1) * 128, :])
                    ps = ps_pro.tile([128, 128], F32, tag="xtp", name="xtp")
                    nc.tensor.transpose(ps[:], xt[:], ident[:])
                    if i % 2 == 0:
                        nc.vector.tensor_copy(xT[:, i * 128:(i + 1) * 128], ps[:])
                    else:
                        nc.scalar.copy(xT[:, i * 128:(i + 1) * 128], ps[:])
                pk = ps_pro.tile([128, 512], F32, tag="proj", name="pk")
                nc.tensor.matmul(pk[:], wk_r[:], xT[:, sl], start=True, stop=True)
                nc.vector.tensor_copy(kT[:, sl], pk[:])
                pq = ps_pro.tile([128, 512], F32, tag="proj", name="pq")
                nc.tensor.matmul(pq[:], wq_r[:], xT[:, sl], start=True, stop=True)
                nc.scalar.copy(qT[:, sl], pq[:])
                for u in range(4):
                    i = c * 4 + u
                    pv = ps_pro.tile([128, 128], F32, tag="vproj", name="pv")
                    nc.tensor.matmul(
                        pv[:], xT[:, i * 128:(i + 1) * 128], wv_r[:],
                        start=True, stop=True,
                    )
                    nc.scalar.copy(vaug[:, i, 0:128], pv[:])

        # ---- main loop pools: 2x3-bank score slots + 2x1-bank AV accum ----
        ps_s = ctx.enter_context(tc.tile_pool(name="ps_s", bufs=2, space="PSUM"))
        ps_av = ctx.enter_context(tc.tile_pool(name="ps_av", bufs=2, space="PSUM"))

        def score_chunk(qsl, off, width):
            s = ps_s.tile([128, CHUNKS[0]], F32, tag="sh", name="sh")
            for k in range(width // 512):
                nc.tensor.matmul(
                    s[:, k * 512:(k + 1) * 512],
                    qsl,
                    kT[:, off + k * 512: off + (k + 1) * 512],
                    start=True,
                    stop=True,
                )
            return s

        def negmax(s, width, tg, pieces=1):
            if pieces == 1:
                nm = stats.tile([128, 1], F32, tag=tg, name="nm")
                nc.vector.reduce_max(nm[:], s[:, 0:width], axis=AX, negate=True)
                return nm
            w = width // pieces
            parts = []
            for p in range(pieces):
                pm = stats.tile([128, 1], F32, tag=f"{tg}p{p}", name="pm")
                nc.vector.reduce_max(
                    pm[:], s[:, p * w:(p + 1) * w], axis=AX, negate=True
                )
                parts.append(pm)
            nm = parts[0]
            for p in range(1, pieces):
                acc = stats.tile([128, 1], F32, tag=f"{tg}a{p}", name="acc")
                nc.vector.tensor_tensor(acc[:], nm[:], parts[p][:], op=OP.min)
                nm = acc
            return nm

        # chunk c0 is exponentiated with its own max (-n0), c1 with -max(c0,c1),
        # c2 with the full row max; AV rescales by gam_a (after kv-tiles 0..11)
        # and gam_b (after kv-tiles 12..23) restore a common exp(-max) scale.
        T_A = CHUNKS[0] // 128   # 12
        T_B = RESCALE_T          # 24
        t1 = None  # (PT, gama, gamb, j): awaiting AVa/AVb
        t2 = None  # (PT, gamb, av, j):   awaiting gb-rescale + tail + norm
        for i in range(NT + 2):
            # A: tile i - first two score chunks and their maxes
            if i < NT:
                qsl = qT[:, i * 128:(i + 1) * 128]
                P = pbuf.tile([128, N], BF16, tag="P", name="P")
                PT = pbuf.tile([128, NT, 128], BF16, tag="PT", name="PT")
                s0 = score_chunk(qsl, 0, CHUNKS[0])
                s1 = score_chunk(qsl, CHUNKS[0], CHUNKS[1])
                n0 = negmax(s0, CHUNKS[0], "n0")
                n1 = negmax(s1, CHUNKS[1], "n1")
                b01 = stats.tile([128, 1], F32, tag="b01", name="b01")
                nc.vector.tensor_tensor(b01[:], n0[:], n1[:], op=OP.min)

            # B1: tile i-1 - AV over kv-tiles 0..11 (exp(-m0)-scaled)
            if t1 is not None:
                PT1, gama1, gamb1, j1 = t1
                av1 = ps_av.tile([128, 129], F32, tag="av", name="av")
                for t in range(T_A):
                    nc.tensor.matmul(
                        av1[:], PT1[:, t, :], vaug[:, t, :],
                        start=(t == 0), stop=False,
                    )

            # C: tile i-2 - gam_b rescale (ScalarE; inputs one iteration old),
            # AV tail, reciprocal
            if t2 is not None:
                PT2, gamb2, av2, j2 = t2
                nc.scalar.activation(av2[:], av2[:], AF.Copy, bias=0.0, scale=gamb2[:])
                for t in range(T_B, NT):
                    nc.tensor.matmul(
                        av2[:], PT2[:, t, :], vaug[:, t, :],
                        start=False, stop=(t == NT - 1),
                    )
                linv = stats.tile([128, 1], F32, tag="linv", name="linv")
                nc.vector.reciprocal(linv[:], av2[:, 128:129])

            # D1: tile i - exp of c0 (own bias), last chunk + stats + gammas
            if i < NT:
                nc.scalar.activation(P[:, 0:CHUNKS[0]], s0[:], AF.Exp, bias=n0[:])
                off2 = CHUNKS[0] + CHUNKS[1]
                s2 = score_chunk(qsl, off2, CHUNKS[2])
                n2 = negmax(s2, CHUNKS[2], "n2", pieces=2)
                bias = stats.tile([128, 1], F32, tag="bias", name="bias")
                nc.vector.tensor_tensor(bias[:], b01[:], n2[:], op=OP.min)
                gina = stats.tile([128, 1], F32, tag="gina", name="gina")
                nc.vector.tensor_tensor(gina[:], b01[:], n0[:], op=OP.subtract)
                gama = stats.tile([128, 1], F32, tag="gama", name="gama")
                nc.scalar.activation(gama[:], gina[:], AF.Exp)
                ginb = stats.tile([128, 1], F32, tag="ginb", name="ginb")
                nc.vector.tensor_tensor(ginb[:], bias[:], b01[:], op=OP.subtract)
                gamb = stats.tile([128, 1], F32, tag="gamb", name="gamb")
                nc.scalar.activation(gamb[:], ginb[:], AF.Exp)

            # B2: tile i-1 - gam_a rescale then AV over kv-tiles 12..23
            if t1 is not None:
                nc.scalar.activation(av1[:], av1[:], AF.Copy, bias=0.0, scale=gama1[:])
                for t in range(T_A, T_B):
                    nc.tensor.matmul(
                        av1[:], PT1[:, t, :], vaug[:, t, :],
                        start=False, stop=False,
                    )

            # D2: tile i - exps of c1/c2, xbar transposes
            if i < NT:
                nc.sync.dma_start_transpose(
                    PT[:, 0:T_A, :], P[:, 0:T_A * 128]
                )
                nc.scalar.activation(
                    P[:, CHUNKS[0]:off2], s1[:, 0:CHUNKS[1]], AF.Exp, bias=b01[:]
                )
                nc.sync.dma_start_transpose(
                    PT[:, T_A:T_B, :], P[:, T_A * 128:T_B * 128]
                )
                nc.scalar.activation(
                    P[:, off2:N], s2[:, 0:CHUNKS[2]], AF.Exp, bias=bias[:]
                )
                nc.sync.dma_start_transpose(
                    PT[:, T_B:NT, :], P[:, T_B * 128:N]
                )

            # E: tile i-2 - normalize and store
            if t2 is not None:
                ost = ostage.tile([128, 128], F32, tag="ost", name="ost")
                nc.scalar.activation(
                    ost[:], av2[:, 0:128], AF.Copy, bias=0.0, scale=linv[:]
                )
                nc.gpsimd.dma_start(out[j2 * 128:(j2 + 1) * 128, :], ost[:])

            t2 = (t1[0], t1[2], av1, t1[3]) if t1 is not None else None
            t1 = (PT, gama, gamb, i) if i < NT else None

    nc.compile()
    return nc


_NC_CACHE = {}


def _get_nc():
    if "nc" not in _NC_CACHE:
        nc = bacc.Bacc("TRN2", target_bir_lowering=False, debug=False, num_devices=B)
        _NC_CACHE["nc"] = build_attention(nc)
    return _NC_CACHE["nc"]


def kernel(x, w_query, w_key, w_value, _trace=False):
    x = np.ascontiguousarray(np.asarray(x, dtype=np.float32))
    w_query = np.ascontiguousarray(np.asarray(w_query, dtype=np.float32))
    w_key = np.ascontiguousarray(np.asarray(w_key, dtype=np.float32))
    w_value = np.ascontiguousarray(np.asarray(w_value, dtype=np.float32))
    nc = _get_nc()
    in_maps = [
        {"x": x[b], "w_query": w_query, "w_key": w_key, "w_value": w_value}
        for b in range(B)
    ]
    res = run_bass_kernel_spmd(nc, in_maps, core_ids=list(range(B)), trace=_trace)
    out_full = np.stack([res.results[b]["out"] for b in range(B)])
    if _trace:
        kernel.last_exec_time_ns = res.exec_time_ns
    return out_full
